# revision 4
# baseline (speedup 1.0000x reference)
"""Clustered (k-means routed) attention on 8 NeuronCores.

Sharding: data-parallel over batch (B=8 -> 1 batch element per core),
QKV/unify weights and cluster means replicated. Each core runs the full
per-batch clustered-attention pipeline; outputs are gathered on host.

The program is split into two pmapped stages at the scatter boundary —
a single fused graph triggers a runtime INTERNAL error in the neuron
backend at the scatter-add when it is fused with the upstream attention;
as separate executables both run fine. Intermediates (bo, qi) stay
device-resident between the two pmap calls.
"""

import numpy as np
import jax
import jax.numpy as jnp

# Exact fp32 matmuls: top-k membership is decided by distance comparisons
# whose ~1e-4 gaps would be swamped by bf16-downcast matmuls.
jax.config.update("jax_default_matmul_precision", "highest")

B, C, E = 8, 4096, 512
H, D = 4, 128
NC, WSZ = 64, 128
COMMITMENT = 1e-4


def _part1(x, Wq, bq, Wk, bk, Wv, bv, means):
    # x: [C, E] one batch element on one core
    q = (x @ Wq + bq).reshape(C, H, D).transpose(1, 0, 2)  # [h, c, d]
    k = (x @ Wk + bk).reshape(C, H, D).transpose(1, 0, 2)
    v = (x @ Wv + bv).reshape(C, H, D).transpose(1, 0, 2)

    qk = jnp.concatenate([q, k], axis=1)  # [h, 2c, d]
    xn = qk / jnp.maximum(jnp.linalg.norm(qk, axis=-1, keepdims=True), 1e-12)
    dists = jnp.einsum("hld,hcd->hlc", xn, means)  # [h, 2c, nc]

    # aux_loss identity: routed = means[argmax] is unit-norm and ||xn|| = 1,
    # so sum_d (xn - routed)^2 = 2 - 2 * max_c dists. Avoids argmax+gather.
    aux_part = jnp.mean(2.0 - 2.0 * jnp.max(dists, axis=-1))

    tk = lambda dd: jax.lax.top_k(dd.transpose(0, 2, 1), WSZ)[1].reshape(H, NC * WSZ)
    qi = tk(dists[:, :C])
    ki = tk(dists[:, C:])

    g = lambda t, i: jnp.take_along_axis(t, i[..., None], axis=1)
    qs = g(q, qi).reshape(H, NC, WSZ, D)
    ks = g(k, ki).reshape(H, NC, WSZ, D)
    vs = g(v, ki).reshape(H, NC, WSZ, D)

    dots = jnp.einsum("hnid,hnjd->hnij", qs, ks) * (D ** -0.5)
    attn = jax.nn.softmax(dots, axis=-1)
    bo = jnp.einsum("hnij,hnjd->hnid", attn, vs).reshape(H, NC * WSZ, D)
    return bo, qi, aux_part


def _part2(bo, qi, Wu, bu):
    flat = (jnp.arange(H)[:, None] * C + qi).reshape(-1)
    numer = jnp.zeros((H * C, D), jnp.float32).at[flat].add(bo.reshape(-1, D))
    denom = jnp.zeros((H * C, D), jnp.float32).at[flat].add(
        jnp.ones((H * NC * WSZ, D), jnp.float32)
    )
    out_h = (numer / (denom + 1e-5)).reshape(H, C, D)
    return out_h.transpose(1, 0, 2).reshape(C, H * D) @ Wu + bu


_fns = None


def _get_fns():
    global _fns
    if _fns is None:
        devs = jax.devices()[:B]
        p1 = jax.pmap(_part1, in_axes=(0,) + (None,) * 7, devices=devs)
        p2 = jax.pmap(_part2, in_axes=(0, 0, None, None), devices=devs)
        _fns = (p1, p2)
    return _fns


def kernel(x, attention_mask, Wq, bq, Wk, bk, Wv, bv, Wu, bu, means):
    del attention_mask  # accepted but unused (matches reference forward)
    p1, p2 = _get_fns()
    bo, qi, aux_parts = p1(
        jnp.asarray(x, jnp.float32),
        jnp.asarray(Wq), jnp.asarray(bq),
        jnp.asarray(Wk), jnp.asarray(bk),
        jnp.asarray(Wv), jnp.asarray(bv),
        jnp.asarray(means),
    )
    out = p2(bo, qi, jnp.asarray(Wu), jnp.asarray(bu))
    out = np.asarray(out, np.float32)
    # aux_loss = COMMITMENT * mean over all (b,h,2c,d); per-b partials have
    # equal weight so the host-side mean is exact.
    aux = np.float32(np.mean(np.asarray(aux_parts, np.float64)) * COMMITMENT)
    return out, aux


# revision 6
# speedup vs baseline: 1.1314x; 1.1314x over previous
"""Clustered (k-means routed) attention on 8 NeuronCores.

Sharding: data-parallel over batch (B=8 -> 1 batch element per core),
QKV/unify weights and cluster means replicated (device-resident, cached
across calls). Each core runs the full per-batch clustered-attention
pipeline; outputs are gathered on host.

The program is split into two pmapped stages at the scatter boundary —
a single fused graph triggers a runtime INTERNAL error in the neuron
backend at the scatter-add when fused with the upstream attention; as
separate executables both run fine. Intermediates (bo, qi) stay
device-resident between the two pmap calls.
"""

import numpy as np
import jax
import jax.numpy as jnp

# Exact fp32 matmuls: top-k membership is decided by distance comparisons
# whose ~1e-4 gaps would be swamped by bf16-downcast matmuls.
jax.config.update("jax_default_matmul_precision", "highest")

B, C, E = 8, 4096, 512
H, D = 4, 128
NC, WSZ = 64, 128
COMMITMENT = 1e-4


def _part1(x, Wq, bq, Wk, bk, Wv, bv, means):
    # x: [C, E] one batch element on one core
    q = (x @ Wq + bq).reshape(C, H, D).transpose(1, 0, 2)  # [h, c, d]
    k = (x @ Wk + bk).reshape(C, H, D).transpose(1, 0, 2)
    v = (x @ Wv + bv).reshape(C, H, D).transpose(1, 0, 2)

    qk = jnp.concatenate([q, k], axis=1)  # [h, 2c, d]
    xn = qk / jnp.maximum(jnp.linalg.norm(qk, axis=-1, keepdims=True), 1e-12)
    dists = jnp.einsum("hld,hcd->hlc", xn, means)  # [h, 2c, nc]

    # aux_loss identity: routed = means[argmax] is unit-norm and ||xn|| = 1,
    # so sum_d (xn - routed)^2 = 2 - 2 * max_c dists; the reference mean also
    # runs over the d axis, hence the / D.
    aux_part = jnp.mean(2.0 - 2.0 * jnp.max(dists, axis=-1)) / D

    tk = lambda dd: jax.lax.top_k(dd.transpose(0, 2, 1), WSZ)[1].reshape(H, NC * WSZ)
    qi = tk(dists[:, :C])
    ki = tk(dists[:, C:])

    g = lambda t, i: jnp.take_along_axis(t, i[..., None], axis=1)
    qs = g(q, qi).reshape(H, NC, WSZ, D)
    ks = g(k, ki).reshape(H, NC, WSZ, D)
    vs = g(v, ki).reshape(H, NC, WSZ, D)

    dots = jnp.einsum("hnid,hnjd->hnij", qs, ks) * (D ** -0.5)
    attn = jax.nn.softmax(dots, axis=-1)
    bo = jnp.einsum("hnij,hnjd->hnid", attn, vs).reshape(H, NC * WSZ, D)
    return bo, qi, aux_part


def _part2(bo, qi, Wu, bu):
    flat = (jnp.arange(H)[:, None] * C + qi).reshape(-1)
    numer = jnp.zeros((H * C, D), jnp.float32).at[flat].add(bo.reshape(-1, D))
    denom = jnp.zeros((H * C, D), jnp.float32).at[flat].add(
        jnp.ones((H * NC * WSZ, D), jnp.float32)
    )
    out_h = (numer / (denom + 1e-5)).reshape(H, C, D)
    return out_h.transpose(1, 0, 2).reshape(C, H * D) @ Wu + bu


_state = None


def _get_state(weights):
    """Compile pmapped stages and replicate weights onto the 8 cores once."""
    global _state
    key = tuple(id(w) for w in weights)
    if _state is not None and _state[0] == key:
        return _state[1], _state[2], _state[3]
    devs = jax.devices()[:B]
    p1 = jax.pmap(_part1, devices=devs)          # all args sharded/replicated
    p2 = jax.pmap(_part2, devices=devs)
    wrep = [
        jax.device_put_replicated(np.asarray(w, np.float32), devs) for w in weights
    ]
    _state = (key, p1, p2, wrep)
    return p1, p2, wrep


def kernel(x, attention_mask, Wq, bq, Wk, bk, Wv, bv, Wu, bu, means):
    del attention_mask  # accepted but unused (matches reference forward)
    p1, p2, (Wq_, bq_, Wk_, bk_, Wv_, bv_, Wu_, bu_, means_) = _get_state(
        (Wq, bq, Wk, bk, Wv, bv, Wu, bu, means)
    )
    xs = np.ascontiguousarray(np.asarray(x, np.float32))
    bo, qi, aux_parts = p1(xs, Wq_, bq_, Wk_, bk_, Wv_, bv_, means_)
    out = p2(bo, qi, Wu_, bu_)
    out = np.asarray(out, np.float32)
    # aux_loss = COMMITMENT * mean over all (b,h,2c,d); per-b partials have
    # equal weight so the host-side mean is exact.
    aux = np.float32(np.mean(np.asarray(aux_parts, np.float64)) * COMMITMENT)
    return out, aux
